# revision 3
# baseline (speedup 1.0000x reference)
"""4-layer GAT (GNN message passing) on 8 TRN2 NeuronCores — v6.

- Degree-balanced node blocks (49/core); per-layer fused loop:
  edge-aggregate(li) then node-project(li+1); h lives in SBUF (bf16).
- Node table AllGathered in THREE chunks per layer (blocks 0-19, 20-38,
  39-48) so most AG transfer overlaps compute; chunk tables are int16-
  indexable for dma_gather.
- Edge gathers: gpsimd dma_gather, <=1024 indices (8 chunks) per
  instruction, variable per-block chunk counts (no global-max padding).
- al_dst per edge via PE expansion: one-hot transposed by the DMA xbar,
  small matmul against the block's al_dst vector (no per-edge dst gather).
- z transposed for the projection matmul by ONE dma xbar transpose per
  block (chunk-major layout) instead of PE transposes + PSUM copies.
- One-hot built on-chip (iota + is_equal), bf16 matmuls, attention-logit
  weights folded into the projection matmul, GAT bias folded into the
  table (softmax weights sum to 1).
"""
import heapq
import ml_dtypes
import numpy as np

from concourse import bass, bacc, mybir, tile
from concourse.bass_utils import run_bass_kernel_spmd
from concourse.masks import make_identity

F32 = mybir.dt.float32
BF16 = mybir.dt.bfloat16
I16 = mybir.dt.int16
AF = mybir.ActivationFunctionType
AX = mybir.AxisListType
OP = mybir.AluOpType

N = 50000
IN = 128
HID = 64
HEADS = 8
OUT = 3
NEG_SLOPE = 0.2
EPS = 1e-6
P = 128
NCORES = 8
NB = 49
SPLITS = (20, 19, 10)      # blocks per AG chunk (3 chunks per layer)
NPC = NB * P
NTOT = NCORES * NPC
EB = 3                     # dst blocks per gather group
TCW = 640                  # padded table row (1280B)
TC3 = 128                  # padded layer-3 table row (256B)

LAYERS = [(IN, 512, 8, 64), (512, 512, 8, 64), (512, 512, 8, 64), (512, 3, 1, 3)]

SPLIT_BASE = (0, SPLITS[0], SPLITS[0] + SPLITS[1])


def _split_of(t):
    return 0 if t < SPLIT_BASE[1] else (1 if t < SPLIT_BASE[2] else 2)


# ----------------------------------------------------------------- host prep

def _wrap_idx(flat):
    flat = np.asarray(flat, np.int16)
    assert len(flat) % 16 == 0
    w = flat.reshape(-1, 16).T
    return np.tile(w, (8, 1))


def _build_partition(edge_index):
    src0 = edge_index[0].astype(np.int64)
    dst0 = edge_index[1].astype(np.int64)
    loops = np.arange(N, dtype=np.int64)
    src = np.concatenate([src0, loops])
    dst = np.concatenate([dst0, loops])
    deg = np.bincount(dst, minlength=N)

    nblocks = NCORES * NB
    order = np.argsort(-deg, kind="stable")
    heap = [(0, 0, b) for b in range(nblocks)]
    heapq.heapify(heap)
    blk_of = np.empty(N, np.int64)
    slot_of = np.empty(N, np.int64)
    spill = []
    for v in order:
        while True:
            load, cnt, b = heapq.heappop(heap)
            if cnt < P:
                break
            spill.append((load, cnt, b))
        blk_of[v] = b
        slot_of[v] = cnt
        heapq.heappush(heap, (load + int(deg[v]), cnt + 1, b))
        for item in spill:
            heapq.heappush(heap, item)
        spill.clear()

    new_id = blk_of * P + slot_of
    filled = np.zeros(NTOT, bool)
    filled[new_id] = True
    pad_ids = np.nonzero(~filled)[0]

    nsrc = new_id[src]
    ndst = new_id[dst]
    nsrc = np.concatenate([nsrc, np.zeros(len(pad_ids), np.int64)])
    ndst = np.concatenate([ndst, pad_ids])

    # src split + row within that split's gathered table
    s_core = nsrc // NPC
    s_t = (nsrc % NPC) // P
    s_p = nsrc % P
    s_spl = np.where(s_t < SPLIT_BASE[1], 0, np.where(s_t < SPLIT_BASE[2], 1, 2))
    spl_sz = np.array(SPLITS)[s_spl]
    spl_b = np.array(SPLIT_BASE)[s_spl]
    s_row = (s_core * spl_sz + (s_t - spl_b)) * P + s_p

    eblk = ndst // P
    order_e = np.lexsort((ndst, s_spl, eblk))
    nsrc, ndst, eblk = nsrc[order_e], ndst[order_e], eblk[order_e]
    s_spl, s_row = s_spl[order_e], s_row[order_e]

    cnt = np.zeros((nblocks, 3), np.int64)
    np.add.at(cnt, (eblk, s_spl), 1)
    nch = (cnt + P - 1) // P                       # chunks per (block, split)

    # per-core chunk layout: block-major, within block split0|split1|split2
    # (same order for the one-hot table and the gather streams per split)
    sidx = [[], [], []]        # flat per split: gather stream indices
    dl_cols = []               # [P] columns, block-major over all chunks
    starts = np.concatenate([[0], np.cumsum(cnt.reshape(-1))])
    percore = []
    for c in range(NCORES):
        n0 = nch[c * NB:(c + 1) * NB]              # [NB, 3]
        percore.append(n0.copy())
    for b in range(nblocks):
        c, bl = divmod(b, NB)
        for spl in range(3):
            k = int(cnt[b, spl])
            es = starts[b * 3 + spl]
            rows = s_row[es:es + k]
            bd = ndst[es:es + k] - b * P
            for ch in range(int(nch[b, spl])):
                lo, hi = ch * P, min((ch + 1) * P, k)
                n = hi - lo
                si = np.zeros(P, np.int16)
                si[:n] = rows[lo:hi]
                sidx[spl].append((c, si))
                dc = np.full(P, -1.0, np.float32)
                dc[:n] = bd[lo:hi]
                dl_cols.append((c, dc))
    # assemble per-core tables
    sidx_t = []
    for spl in range(3):
        per = [np.concatenate([s for cc, s in sidx[spl] if cc == c])
               if any(cc == c for cc, s in sidx[spl]) else np.zeros(0, np.int16)
               for c in range(NCORES)]
        sidx_t.append(per)
    dl_t = [np.stack([d for cc, d in dl_cols if cc == c], axis=1)
            for c in range(NCORES)]               # [P, nchunks_c]
    return new_id, sidx_t, dl_t, percore


def _fold_weights(inputs):
    ws = {}
    for i, (ci, D, h, co) in enumerate(LAYERS):
        g = np.asarray(inputs[f'ln_g{i}'], np.float64)
        b = np.asarray(inputs[f'ln_b{i}'], np.float64)
        W = np.asarray(inputs[f'W{i}'], np.float64)
        asr = np.asarray(inputs[f'asr{i}'], np.float64)
        adt = np.asarray(inputs[f'adt{i}'], np.float64)
        bias = np.asarray(inputs[f'b{i}'], np.float64)
        wp = (W * g[None, :]).T
        cv0 = W @ b
        wa_s = np.einsum('chk,hk->ch', wp.reshape(ci, h, co), asr)
        wa_d = np.einsum('chk,hk->ch', wp.reshape(ci, h, co), adt)
        cva_s = (cv0.reshape(h, co) * asr).sum(-1)
        cva_d = (cv0.reshape(h, co) * adt).sum(-1)
        cvm = cv0 + bias
        kc = ci // P
        if i < 3:
            wm = wp.reshape(kc, P, D).transpose(1, 0, 2).reshape(P, kc * D)
            wx = np.concatenate([wa_s, wa_d], 1)
            wxr = wx.reshape(kc, P, 2 * h).transpose(1, 0, 2).reshape(P, kc * 2 * h)
            ws[f'wm{i}'] = wm.astype(ml_dtypes.bfloat16)
            ws[f'wx{i}'] = wxr.astype(ml_dtypes.bfloat16)
            ws[f'cvm{i}'] = cvm.reshape(1, D).astype(np.float32)
            ws[f'cvx{i}'] = np.concatenate([cva_s, cva_d]).reshape(1, 2 * h).astype(np.float32)
        else:
            wcat = np.concatenate([wp, wa_s, wa_d], 1)
            wmr = wcat.reshape(kc, P, 5).transpose(1, 0, 2).reshape(P, kc * 5)
            ws['wm3'] = wmr.astype(ml_dtypes.bfloat16)
            ws['cv3'] = np.concatenate([cvm, cva_s, cva_d]).reshape(1, 5).astype(np.float32)
    ws['res_wt'] = np.ascontiguousarray(
        np.asarray(inputs['res_W'], np.float32).T)
    ws['res_b'] = np.asarray(inputs['res_b'], np.float32).reshape(1, OUT)
    return ws


# -------------------------------------------------------------- bass program

def _build_program(nch):
    """nch: [NB, 3] chunks per (block, split) for this core layout
    (identical across cores by construction of the tables? NO — per-core).
    """
    raise NotImplementedError


def _build_program_percore(nch_all):
    # nch_all: [NCORES][NB, 3] — per-core chunk counts. SPMD requires ONE
    # program: use per-core MAX? No — the program must be identical across
    # cores. Use the elementwise max over cores so every core runs the same
    # instruction stream; per-core tables pad the extra chunks (dl=-1).
    nch = np.maximum.reduce([np.asarray(x) for x in nch_all])   # [NB, 3]
    return nch


_CACHE = {}
_LAST_RES = None


def _program(nch):
    key = tuple(map(tuple, nch.tolist()))
    if key in _CACHE:
        return _CACHE[key]

    nc = bacc.Bacc("TRN2", target_bir_lowering=False, debug=False,
                   num_devices=NCORES)
    nblk = [int(x) for x in nch.sum(0)]            # chunks per split stream
    ctb = nch.sum(1).astype(int)                   # chunks per block
    NCH = int(ctb.sum())
    base_s = [np.concatenate([[0], np.cumsum(nch[:, s])]).astype(int)
              for s in range(3)]                   # chunk base per split
    base_b = np.concatenate([[0], np.cumsum(ctb)]).astype(int)

    x_s = nc.dram_tensor("x_s", [NPC, IN], F32, kind="ExternalInput")
    sidx_d = [nc.dram_tensor(f"sidx{s}", [P, nblk[s] * 8], I16,
                             kind="ExternalInput") for s in range(3)]
    dl_d = nc.dram_tensor("dl", [P, NCH], F32, kind="ExternalInput")
    wdram = {}
    for i, (ci, D, h, co) in enumerate(LAYERS):
        kc = ci // P
        if i < 3:
            wdram[f'wm{i}'] = nc.dram_tensor(f"wm{i}", [P, kc * D], BF16, kind="ExternalInput")
            wdram[f'wx{i}'] = nc.dram_tensor(f"wx{i}", [P, kc * 2 * h], BF16, kind="ExternalInput")
            wdram[f'cvm{i}'] = nc.dram_tensor(f"cvm{i}", [1, D], F32, kind="ExternalInput")
            wdram[f'cvx{i}'] = nc.dram_tensor(f"cvx{i}", [1, 2 * h], F32, kind="ExternalInput")
        else:
            wdram['wm3'] = nc.dram_tensor("wm3", [P, kc * 5], BF16, kind="ExternalInput")
            wdram['cv3'] = nc.dram_tensor("cv3", [1, 5], F32, kind="ExternalInput")
    wdram['res_wt'] = nc.dram_tensor("res_wt", [IN, OUT], F32, kind="ExternalInput")
    wdram['res_b'] = nc.dram_tensor("res_b", [1, OUT], F32, kind="ExternalInput")
    out_ext = nc.dram_tensor("out", [NPC, OUT], F32, kind="ExternalOutput")

    rg = [list(range(NCORES))]

    with tile.TileContext(nc) as tc:
        with (
            tc.tile_pool(name="dram", bufs=1, space="DRAM") as dpool,
            tc.tile_pool(name="const", bufs=1) as cpool,
            tc.tile_pool(name="persist", bufs=1) as hpool,
            tc.tile_pool(name="node", bufs=2) as npool,
            tc.tile_pool(name="stats", bufs=3) as spool,
            tc.tile_pool(name="edge", bufs=2) as epool,
            tc.tile_pool(name="bigA", bufs=2) as bpoolA,
            tc.tile_pool(name="bigB", bufs=2) as bpoolB,
            tc.tile_pool(name="bigC", bufs=2) as bpoolC,
            tc.tile_pool(name="st", bufs=2) as stpool,
            tc.tile_pool(name="psA", bufs=2, space="PSUM") as psA,
            tc.tile_pool(name="psP", bufs=2, space="PSUM") as psP,
            tc.tile_pool(name="psT", bufs=2, space="PSUM") as psT,
            tc.tile_pool(name="psX", bufs=2, space="PSUM") as psX,
        ):
            # --- DRAM intermediates
            shard = {}
            full = {}
            for li in range(4):
                tcw = TCW if li < 3 else TC3
                for s in range(3):
                    shard[(li, s)] = dpool.tile([SPLITS[s] * P, tcw], BF16,
                                                name=f"shard{li}_{s}")
                    full[(li, s)] = dpool.tile([NCORES * SPLITS[s] * P, tcw],
                                               BF16, name=f"full{li}_{s}",
                                               addr_space="Shared")

            # --- constants
            identf = cpool.tile([P, P], F32, name="identf")
            make_identity(nc, identf[:])
            iota_t = cpool.tile([P, P], F32, name="iota_t")
            nc.gpsimd.iota(iota_t[:], pattern=[[1, P]], base=0,
                           channel_multiplier=0,
                           allow_small_or_imprecise_dtypes=True)
            sidx = []
            for s in range(3):
                t = cpool.tile([P, nblk[s] * 8], I16, name=f"sidx{s}")
                nc.sync.dma_start(out=t[:], in_=sidx_d[s][:])
                sidx.append(t)
            dl = cpool.tile([P, NCH], F32, name="dl")
            nc.sync.dma_start(out=dl[:], in_=dl_d[:])
            wsb = {}
            for i, (ci, D, h, co) in enumerate(LAYERS):
                kc = ci // P
                if i < 3:
                    for nm, cols in ((f'wm{i}', kc * D), (f'wx{i}', kc * 2 * h)):
                        t = cpool.tile([P, cols], BF16, name=f"{nm}sb")
                        nc.sync.dma_start(out=t[:], in_=wdram[nm][:])
                        wsb[nm] = t
                    for nm, cols in ((f'cvm{i}', D), (f'cvx{i}', 2 * h)):
                        t = cpool.tile([P, cols], F32, name=f"{nm}sb")
                        nc.sync.dma_start(out=t[:], in_=wdram[nm][:].to_broadcast([P, cols]))
                        wsb[nm] = t
                else:
                    t = cpool.tile([P, 4 * 5], BF16, name="wm3sb")
                    nc.sync.dma_start(out=t[:], in_=wdram['wm3'][:])
                    wsb['wm3'] = t
                    t = cpool.tile([P, 5], F32, name="cv3sb")
                    nc.sync.dma_start(out=t[:], in_=wdram['cv3'][:].to_broadcast([P, 5]))
                    wsb['cv3'] = t
            res_wt = cpool.tile([IN, OUT], F32, name="res_wt_sb")
            nc.sync.dma_start(out=res_wt[:], in_=wdram['res_wt'][:])
            res_b = cpool.tile([P, OUT], F32, name="res_b_sb")
            nc.sync.dma_start(out=res_b[:],
                              in_=wdram['res_b'][:].to_broadcast([P, OUT]))
            eps_t = cpool.tile([P, 1], F32, name="eps_t")
            nc.gpsimd.memset(eps_t[:], EPS)

            # --- persistent state
            h_sb = hpool.tile([P, NB * 512], BF16, name="h_sb")
            res_sb = hpool.tile([P, NB * OUT], F32, name="res_sb")
            adst_sb = [hpool.tile([P, NB * 8], BF16, name=f"adst_sb{pq}")
                       for pq in range(2)]

            # ---------------- node projection ----------------
            def node_block(li, t):
                ci, D, h, co = LAYERS[li]
                kc = ci // P
                smn = psX.tile([P, 256], F32, name="smn", tag="sm")[:]
                s = _split_of(t)
                srows = slice((t - SPLIT_BASE[s]) * P, (t - SPLIT_BASE[s] + 1) * P)
                arows = slice(t * P, (t + 1) * P)
                if li == 0:
                    ht = npool.tile([P, ci], F32, name="ht", tag="ht")
                    nc.sync.dma_start(out=ht[:], in_=x_s[arows, :])
                    hin = ht[:]
                else:
                    hin = h_sb[:, t * 512:(t + 1) * 512]

                mu = spool.tile([P, 1], F32, name="mu", tag="st1")
                nc.vector.reduce_sum(out=mu[:], in_=hin, axis=AX.X)
                nc.vector.tensor_scalar_mul(mu[:], mu[:], -1.0 / ci)
                sq = npool.tile([P, ci], BF16, name="sq", tag="sq")
                ssq = spool.tile([P, 1], F32, name="ssq", tag="st2")
                nc.scalar.activation(out=sq[:], in_=hin, func=AF.Square,
                                     bias=mu[:, :1], accum_out=ssq[:, :1])
                std = spool.tile([P, 1], F32, name="std", tag="st3")
                nc.scalar.activation(out=std[:], in_=ssq[:], func=AF.Sqrt,
                                     scale=1.0 / ci, bias=eps_t[:, :1])
                rstd = spool.tile([P, 1], F32, name="rstd", tag="st4")
                nc.vector.reciprocal(out=rstd[:], in_=std[:])
                mur = spool.tile([P, 1], F32, name="mur", tag="st5")
                nc.vector.tensor_tensor(out=mur[:], in0=mu[:], in1=rstd[:],
                                        op=OP.mult)
                z = npool.tile([P, ci], BF16, name="z", tag="z")
                nc.scalar.activation(out=z[:], in_=hin, func=AF.Identity,
                                     scale=rstd[:, :1], bias=mur[:, :1])
                # transpose z (chunk-major) on the DMA xbar
                zt = npool.tile([P, ci], BF16, name="zt", tag="zt")
                if kc > 1:
                    nc.sync.dma_start_transpose(
                        out=zt[:].rearrange("p (k f) -> p k f", k=kc), in_=z[:])
                else:
                    nc.sync.dma_start_transpose(out=zt[:], in_=z[:])

                if li == 0:
                    xt_ps = psT.tile([P, P], F32, name="xt_ps", tag="tp")
                    nc.tensor.transpose(out=xt_ps[:], in_=ht[:], identity=identf[:])
                    xt_sb = npool.tile([P, P], F32, name="xt_sb", tag="xts")
                    nc.scalar.copy(out=xt_sb[:], in_=xt_ps[:])
                    res_ps = smn[:, 32:48]
                    nc.tensor.matmul(out=res_ps[:, :OUT], lhsT=xt_sb[:],
                                     rhs=res_wt[:], start=True, stop=True)
                    nc.vector.tensor_tensor(
                        out=res_sb[:, t * OUT:(t + 1) * OUT],
                        in0=res_ps[:, :OUT], in1=res_b[:], op=OP.add)

                if li < 3:
                    pp = psP.tile([P, 512], F32, name="pp", tag="pp")
                    px = smn[:, 0:16]
                    wm = wsb[f'wm{li}']
                    wx = wsb[f'wx{li}']
                    for k in range(kc):
                        zk = zt[:, k * P:(k + 1) * P]
                        nc.tensor.matmul(out=pp[:, :D], lhsT=zk,
                                         rhs=wm[:, k * D:(k + 1) * D],
                                         start=(k == 0), stop=(k == kc - 1))
                        nc.tensor.matmul(out=px[:, :2 * h], lhsT=zk,
                                         rhs=wx[:, k * 2 * h:(k + 1) * 2 * h],
                                         start=(k == 0), stop=(k == kc - 1))
                    ptile = npool.tile([P, D + h], BF16, name="ptile", tag="pt")
                    nc.vector.tensor_tensor(out=ptile[:, 0:D], in0=pp[:, :D],
                                            in1=wsb[f'cvm{li}'][:], op=OP.add)
                    nc.vector.tensor_tensor(out=ptile[:, D:D + h],
                                            in0=px[:, 0:h],
                                            in1=wsb[f'cvx{li}'][:, 0:h], op=OP.add)
                    nc.vector.tensor_tensor(
                        out=adst_sb[li % 2][:, t * 8:t * 8 + h],
                        in0=px[:, h:2 * h],
                        in1=wsb[f'cvx{li}'][:, h:2 * h], op=OP.add)
                    nc.sync.dma_start(out=shard[(li, s)][srows, 0:D + h],
                                      in_=ptile[:])
                else:
                    pp = smn[:, 64:80]
                    wm = wsb['wm3']
                    for k in range(kc):
                        zk = zt[:, k * P:(k + 1) * P]
                        nc.tensor.matmul(out=pp[:, :5], lhsT=zk,
                                         rhs=wm[:, k * 5:(k + 1) * 5],
                                         start=(k == 0), stop=(k == kc - 1))
                    ptile = npool.tile([P, 4], BF16, name="ptile3", tag="pt3")
                    nc.vector.tensor_tensor(out=ptile[:, 0:4], in0=pp[:, :4],
                                            in1=wsb['cv3'][:, 0:4], op=OP.add)
                    nc.vector.tensor_tensor(
                        out=adst_sb[3 % 2][:, t * 8:t * 8 + 1],
                        in0=pp[:, 4:5],
                        in1=wsb['cv3'][:, 4:5], op=OP.add)
                    nc.sync.dma_start(out=shard[(3, s)][srows, 0:4], in_=ptile[:])

            def ag_split(li, s):
                nc.gpsimd.collective_compute(
                    "AllGather", OP.bypass, replica_groups=rg,
                    ins=[shard[(li, s)][:].opt()],
                    outs=[full[(li, s)][:].opt()])

            # ---------------- edge phase ----------------
            def _wgather(tile_, table, idxtab, ch0, nchk, elem):
                w = 0
                while w < nchk:
                    n = min(8, nchk - w)
                    nc.gpsimd.dma_gather(
                        out_ap=tile_[:, w * elem:(w + n) * elem].rearrange(
                            "p (c e) -> p c e", e=elem),
                        in_ap=table[:],
                        idxs_ap=idxtab[:, (ch0 + w) * 8:(ch0 + w + n) * 8],
                        num_idxs=n * P, num_idxs_reg=n * P,
                        elem_size=elem)
                    w += n

            def edge_group(li, gi):
                tcw = TCW if li < 3 else TC3
                b0 = gi * EB
                b1 = min(NB, b0 + EB)
                g = []
                for s, pool_ in ((0, bpoolA), (1, bpoolB), (2, bpoolC)):
                    ch0 = base_s[s][b0]
                    nchk = base_s[s][b1] - ch0
                    t = pool_.tile([P, max(1, nchk) * tcw], BF16,
                                   name=f"g{s}", tag=f"g{s}")
                    if nchk:
                        _wgather(t, full[(li, s)], sidx[s], ch0, nchk, tcw)
                    g.append((t, ch0))
                return g, b0, b1

            def edge_block(li, bl, g):
                ci, D, h, co = LAYERS[li]
                tcw = TCW if li < 3 else TC3
                CT_ = int(ctb[bl])
                nbs = [int(nch[bl, s]) for s in range(3)]
                # per-split views for this block
                segs = []     # (tile, col0, nchunks)
                for s in range(3):
                    t, ch0 = g[s]
                    segs.append((t, (base_s[s][bl] - ch0) * tcw, nbs[s]))

                # one-hot [e, d] per chunk, built in one op per split
                st = stpool.tile([P, CT_ * P], BF16, name="st", tag="st")
                cc0 = 0
                dcol = base_b[bl]
                for s in range(3):
                    if not nbs[s]:
                        continue
                    nc.vector.tensor_tensor(
                        out=st[:, cc0 * P:(cc0 + nbs[s]) * P].rearrange(
                            "p (c d) -> p c d", c=nbs[s]),
                        in0=iota_t[:].unsqueeze(1).broadcast_to([P, nbs[s], P]),
                        in1=dl[:, dcol + cc0:dcol + cc0 + nbs[s]].unsqueeze(
                            2).broadcast_to([P, nbs[s], P]),
                        op=OP.is_equal)
                    cc0 += nbs[s]
                # transposed one-hot via dma xbar (chunk-major)
                stT = stpool.tile([P, CT_ * P], BF16, name="stT", tag="stT")
                nc.sync.dma_start_transpose(
                    out=stT[:].rearrange("p (k f) -> p k f", k=CT_), in_=st[:])
                # al_dst per edge: small matmuls stT_c @ adst_blk
                sme = psX.tile([P, 256], F32, name="sme", tag="sm")[:]
                ald_ps = sme[:, 0:128]
                for cc in range(CT_):
                    nc.tensor.matmul(
                        out=ald_ps[:, cc * h:(cc + 1) * h],
                        lhsT=stT[:, cc * P:(cc + 1) * P],
                        rhs=adst_sb[li % 2][:, bl * 8:bl * 8 + h],
                        start=True, stop=True)
                # e = prelu(al_src + al_dst); ex = exp(e)
                eraw = epool.tile([P, CT_ * h], F32, name="eraw", tag="eraw")
                cc0 = 0
                for s in range(3):
                    t, c0_, nb_ = segs[s]
                    if not nb_:
                        continue
                    nc.vector.tensor_tensor(
                        out=eraw[:, cc0 * h:(cc0 + nb_) * h].rearrange(
                            "p (c h) -> p c h", c=nb_),
                        in0=t[:, c0_:c0_ + nb_ * tcw].rearrange(
                            "p (c t) -> p c t", c=nb_)[:, :, D:D + h],
                        in1=ald_ps[:, cc0 * h:(cc0 + nb_) * h].rearrange(
                            "p (c h) -> p c h", c=nb_),
                        op=OP.add)
                    cc0 += nb_
                el = epool.tile([P, CT_ * h], F32, name="el", tag="el")
                nc.scalar.activation(out=el[:], in_=eraw[:], func=AF.Prelu,
                                     alpha=NEG_SLOPE)
                ex = epool.tile([P, CT_ * h], BF16, name="ex", tag="ex")
                nc.scalar.activation(out=ex[:], in_=el[:], func=AF.Exp)
                # weight messages
                cc0 = 0
                for s in range(3):
                    t, c0_, nb_ = segs[s]
                    if not nb_:
                        continue
                    gv = t[:, c0_:c0_ + nb_ * tcw].rearrange(
                        "p (c t) -> p c t", c=nb_)
                    exv = ex[:, cc0 * h:(cc0 + nb_) * h].rearrange(
                        "p (c h) -> p c h", c=nb_)
                    if li < 3:
                        gm = gv[:, :, 0:D].rearrange("p c (h f) -> p c h f", h=h)
                        nc.vector.tensor_tensor(
                            out=gm, in0=gm,
                            in1=exv.unsqueeze(3).broadcast_to([P, nb_, h, co]),
                            op=OP.mult)
                    else:
                        nc.vector.tensor_tensor(
                            out=gv[:, :, 0:D], in0=gv[:, :, 0:D],
                            in1=exv.broadcast_to([P, nb_, D]), op=OP.mult)
                        nc.scalar.copy(out=gv[:, :, 3:4], in_=exv)
                    cc0 += nb_

                def chunk_rhs(cc, width):
                    for s in range(3):
                        t, c0_, nb_ = segs[s]
                        if cc < nb_:
                            return t[:, c0_ + cc * tcw:c0_ + cc * tcw + width]
                        cc -= nb_
                    raise IndexError

                if li == 3:
                    pa = sme[:, 128:144]
                    for cc in range(CT_):
                        nc.tensor.matmul(
                            out=pa[:, :4], lhsT=st[:, cc * P:(cc + 1) * P],
                            rhs=chunk_rhs(cc, 4),
                            start=(cc == 0), stop=(cc == CT_ - 1))
                    rs = spool.tile([P, 1], F32, name="rs3", tag="rs")
                    nc.vector.reciprocal(out=rs[:], in_=pa[:, 3:4])
                    of = npool.tile([P, OUT], F32, name="of", tag="of")
                    nc.vector.tensor_scalar_mul(of[:], pa[:, 0:3], rs[:, 0:1])
                    nc.vector.tensor_tensor(
                        out=of[:], in0=of[:],
                        in1=res_sb[:, bl * OUT:(bl + 1) * OUT], op=OP.add)
                    rows = slice(bl * P, (bl + 1) * P)
                    nc.sync.dma_start(out=out_ext[rows, :], in_=of[:])
                    return
                pa = psA.tile([P, 512], F32, name="pa", tag="pa")
                pb = sme[:, 128:144]
                for cc in range(CT_):
                    nc.tensor.matmul(
                        out=pa[:, :D], lhsT=st[:, cc * P:(cc + 1) * P],
                        rhs=chunk_rhs(cc, D),
                        start=(cc == 0), stop=(cc == CT_ - 1))
                    nc.tensor.matmul(
                        out=pb[:, :h], lhsT=st[:, cc * P:(cc + 1) * P],
                        rhs=ex[:, cc * h:(cc + 1) * h],
                        start=(cc == 0), stop=(cc == CT_ - 1))
                rs = spool.tile([P, h], F32, name="rs", tag="rs")
                nc.vector.reciprocal(out=rs[:], in_=pb[:, :h])
                gat = npool.tile([P, D], F32, name="gat", tag="gat")
                nc.vector.tensor_tensor(
                    out=gat[:].rearrange("p (h f) -> p h f", h=h),
                    in0=pa[:, :D].rearrange("p (h f) -> p h f", h=h),
                    in1=rs[:].unsqueeze(2).broadcast_to([P, h, co]),
                    op=OP.mult)
                hcols = slice(bl * 512, (bl + 1) * 512)
                if li == 0:
                    nc.scalar.activation(out=h_sb[:, hcols], in_=gat[:],
                                         func=AF.Gelu)
                else:
                    g1 = npool.tile([P, D], BF16, name="g1", tag="g1")
                    nc.scalar.activation(out=g1[:], in_=gat[:], func=AF.Gelu)
                    nc.vector.tensor_tensor(out=h_sb[:, hcols],
                                            in0=h_sb[:, hcols], in1=g1[:],
                                            op=OP.add)

            # ---------------- main flow ----------------
            ag_pts = {SPLIT_BASE[1] - 1: 0, SPLIT_BASE[2] - 1: 1, NB - 1: 2}
            ngroups = (NB + EB - 1) // EB
            for t in range(NB):
                node_block(0, t)
                if t in ag_pts:
                    ag_split(0, ag_pts[t])
            for li in range(3):
                for gi in range(ngroups):
                    g, b0, b1 = edge_group(li, gi)
                    for bl in range(b0, b1):
                        edge_block(li, bl, g)
                        node_block(li + 1, bl)
                        if bl in ag_pts:
                            ag_split(li + 1, ag_pts[bl])
            for gi in range(ngroups):
                g, b0, b1 = edge_group(3, gi)
                for bl in range(b0, b1):
                    edge_block(3, bl, g)

    nc.compile()
    _CACHE[key] = nc
    return nc


def kernel(**inputs):
    global _LAST_RES
    inputs = {k: np.asarray(v) for k, v in inputs.items()}
    new_id, sidx_t, dl_t, percore = _build_partition(inputs['edge_index'])
    ws = _fold_weights(inputs)

    x = np.asarray(inputs['x'], np.float32)
    xp = np.zeros((NTOT, IN), np.float32)
    xp[new_id] = x

    nch = np.maximum.reduce([np.asarray(x_) for x_ in percore])   # [NB, 3]
    nc = _program(nch)

    # re-layout per-core tables padded to the common (max) chunk counts
    nblkc = nch.sum(0).astype(int)
    base_s = [np.concatenate([[0], np.cumsum(nch[:, s])]).astype(int)
              for s in range(3)]
    NCH = int(nch.sum())
    in_maps = []
    for c in range(NCORES):
        own = np.asarray(percore[c])               # [NB, 3]
        sid = [np.zeros(int(nblkc[s]) * P, np.int16) for s in range(3)]
        dlc = np.full((NCH, P), -1.0, np.float32)
        # own flat tables are packed by own counts; re-scatter into padded
        off_own = [0, 0, 0]
        dl_own = dl_t[c]                           # [P, own_total]
        dcol_own = 0
        dcol_pad = 0
        for bl in range(NB):
            for s in range(3):
                k = int(own[bl, s])
                kmax = int(nch[bl, s])
                sid[s][(base_s[s][bl]) * P:(base_s[s][bl] + k) * P] = \
                    sidx_t[s][c][off_own[s] * P:(off_own[s] + k) * P]
                off_own[s] += k
            # dl columns: block-major with per-split ordering
            for s in range(3):
                k = int(own[bl, s])
                kmax = int(nch[bl, s])
                dlc[dcol_pad:dcol_pad + k] = dl_own[:, dcol_own:dcol_own + k].T
                dcol_own += k
                dcol_pad += kmax
        m = dict(
            x_s=np.ascontiguousarray(xp[c * NPC:(c + 1) * NPC]),
            dl=np.ascontiguousarray(dlc.T),
        )
        for s in range(3):
            m[f'sidx{s}'] = _wrap_idx(sid[s])
        m.update(ws)
        in_maps.append(m)

    res = run_bass_kernel_spmd(nc, in_maps, core_ids=list(range(NCORES)))
    _LAST_RES = res
    outs = np.concatenate([r["out"] for r in res.results], axis=0)
    return np.ascontiguousarray(outs[new_id])
